# revision 52
# baseline (speedup 1.0000x reference)
"""GATv2 (3-layer, heads=4/4/1) full-graph kernel for 8 Trainium2 NeuronCores.

Contract: kernel(**inputs) takes the FULL unsharded inputs (as produced by
setup_inputs()) and returns the FULL [64, 64] float32 output.

v3 design (vs. the v1 baseline):
- Distributed phase A: each core transforms only its 6272-node shard
  (xl = x @ Wl, xr = x @ Wr) and scatter-writes xl rows into layer-indexed
  SHARED DRAM tensors (xl_lo/xl_hi halves so gather/scatter indices fit
  int16).  A tiny flag AllGather is the cross-core barrier; gathers are
  WAW-fenced on the flag.  This removes the replicated full-graph phase A
  and the 12.8MB AllGather of node features entirely.
- Per-edge xr is never gathered: xr tiles stay SBUF-resident and are
  expanded per chunk on the PE via one-hot matmuls (u = ohT.T @ xr_tile +
  I.T @ xl_gathered accumulated in PSUM), with leaky-relu applied straight
  from PSUM on ACT.
- One-hot matrices oh [edge, tgt-lane] and ohT [tgt-lane, edge] are
  precomputed on the host in fp8e4m3 (exact for 0/1) and DMAed per group;
  the DVE is_equal generation is gone.
- Features are packed (cw, h)-major inside a layer so the attention-mult,
  score tree-reduction and alpha broadcast all hit the DVE 2x perf mode.
- Scores are reduced with a log2(C) halving tree of 2-byte adds instead of
  a full-rate TensorReduce.
- Softmax without max-shift (scores empirically in [-8, 7]).
- Scatter-sum + softmax denominators via one-hot matmul into PSUM
  (fp8 lhsT x f16 rhs); per-node normalize; final global-mean-pool
  partials via PE, summed and divided on the host.
"""
import os
import numpy as np
import ml_dtypes

import concourse.bacc as bacc
import concourse.mybir as mybir
import concourse.tile as tile
from concourse._compat import get_trn_type
from concourse.bass_utils import run_bass_kernel_spmd

f16 = mybir.dt.float16
f32 = mybir.dt.float32
f8 = mybir.dt.float8e4
i16 = mybir.dt.int16
f8np = ml_dtypes.float8_e4m3

P = 128
N = 50000
E = 800000
NP_ = 50176            # padded nodes = 392 * 128
NT = NP_ // P          # 392 global tiles
CORES = 8
NTC = NT // CORES      # 49 tiles per core
NC_NODES = NTC * P     # 6272 nodes per core
HALF = NP_ // 2        # 25088 rows per shared half
G_GRAPHS = 64
NEG = 0.2
GROUP = 3              # tiles per gather/compute group
GMAX = int(os.environ.get("GAT_GMAX", "1024"))  # max indices per dma_gather
H_L = [4, 4, 1]
W_L = [128, 128, 64]   # xl/value width per layer

_CACHE = {}


def _pack_idx_image(seq):
    """int16 index sequence -> gather/scatter SBUF image [128, len/16]."""
    n = len(seq)
    assert n % 16 == 0
    img = np.asarray(seq, np.int16).reshape(n // 16, 16).T
    return np.tile(img, (8, 1))


def _groups():
    out = []
    gi = 0
    while gi < NTC:
        out.append((gi, min(GROUP, NTC - gi)))
        gi += GROUP
    return out


def _pack_perm(h, c):
    """column permutation: packed[cw*h_n + hh] = natural[hh*c + cw]."""
    perm = np.empty(h * c, np.int64)
    for cw in range(c):
        for hh in range(h):
            perm[cw * h + hh] = hh * c + cw
    return perm


def _preprocess(x, edge_index, batch, params):
    loops = np.arange(N, dtype=np.int64)
    src = np.concatenate([edge_index[0].astype(np.int64), loops])
    tgt = np.concatenate([edge_index[1].astype(np.int64), loops])
    order = np.argsort(tgt, kind="stable")
    srcs, tgts = src[order], tgt[order]
    islo = srcs < HALF

    bounds = np.searchsorted(tgts, np.arange(0, NP_ + 1, P))
    nlo = np.empty(NT, np.int64)
    nhi = np.empty(NT, np.int64)
    for t in range(NT):
        s, e = bounds[t], bounds[t + 1]
        nlo[t] = int(islo[s:e].sum())
        nhi[t] = (e - s) - nlo[t]
    CH = int(max(1, -(-max(nlo.max(), nhi.max()) // P)))
    CT = 2 * CH

    # per-layer packed weights / attention
    wlrs, att_reps = [], []
    prev_perm = None  # input-feature permutation (packing of previous layer)
    for li, (Wl, Wr, att) in enumerate(params):
        h, c = att.shape
        hc = h * c
        Wl = np.asarray(Wl, np.float32)
        Wr = np.asarray(Wr, np.float32)
        if prev_perm is not None:
            Wl = Wl[prev_perm]
            Wr = Wr[prev_perm]
        if li < 2:
            perm = _pack_perm(h, c)
            Wl = Wl[:, perm]
            Wr = Wr[:, perm]
            att_flat = np.asarray(att, np.float32).reshape(-1)[perm]
            prev_perm = perm
        else:
            att_flat = np.asarray(att, np.float32).reshape(-1)
            prev_perm = None
        wlr = np.zeros((P, 256), np.float16)
        wlr[: Wl.shape[0], :hc] = Wl.astype(np.float16)
        wlr[: Wr.shape[0], 128 : 128 + hc] = Wr.astype(np.float16)
        wlrs.append(wlr)
        af = np.zeros(P, np.float16)
        af[:hc] = att_flat.astype(np.float16)
        att_reps.append(np.tile(af[None, :], (P, 1)))

    ident = np.eye(P).astype(f8np)

    in_maps = []
    for c in range(CORES):
        t0 = c * NTC
        base = t0 * P
        # padded per-(tile, stream) slot tables
        xlo = np.zeros((NTC, CH * P), np.int64)
        xhi = np.zeros((NTC, CH * P), np.int64)
        tl_lo = np.full((NTC, CH * P), -1, np.int64)   # tloc or -1 pad
        tl_hi = np.full((NTC, CH * P), -1, np.int64)
        for tt in range(NTC):
            t = t0 + tt
            s, e = bounds[t], bounds[t + 1]
            sl = tgts[s:e] - t * P
            sp = srcs[s:e]
            lo_mask = islo[s:e]
            k = int(lo_mask.sum()); k2 = (e - s) - k
            xlo[tt, :k] = sp[lo_mask]
            tl_lo[tt, :k] = sl[lo_mask]
            xhi[tt, :k2] = sp[~lo_mask] - HALF
            tl_hi[tt, :k2] = sl[~lo_mask]

        # group-stream-major chunk columns
        lo_imgs, hi_imgs = [], []
        oh_cols, ohT_cols = [], []
        for gi, g in _groups():
            lo_imgs.append(_pack_idx_image(xlo[gi:gi + g].reshape(-1)))
            hi_imgs.append(_pack_idx_image(xhi[gi:gi + g].reshape(-1)))
            tl_seq = np.concatenate(
                [tl_lo[gi:gi + g].reshape(-1), tl_hi[gi:gi + g].reshape(-1)])
            nch2 = 2 * g * CH
            tl_mat = tl_seq.reshape(nch2, P)          # [chunk, lane] -> tloc
            oh = np.zeros((P, nch2, P), f8np)         # [lane, chunk, tgt]
            ohT = np.zeros((P, nch2, P), f8np)        # [tgt, chunk, lane]
            ch_i, ln_i = np.nonzero(tl_mat >= 0)
            tl_v = tl_mat[ch_i, ln_i]
            oh[ln_i, ch_i, tl_v] = 1.0
            ohT[tl_v, ch_i, ln_i] = 1.0
            oh_cols.append(oh)
            ohT_cols.append(ohT)

        # scatter index images (natural rows; token i = node base+i).
        # The off-half call dumps into the 128 pad rows past HALF so both
        # calls keep a full static num_idxs_reg (SPMD: same IR on all cores).
        rows = base + np.arange(NC_NODES)
        dump = HALF + (np.arange(NC_NODES) % P)
        sxlo = _pack_idx_image(rows if base < HALF else dump)
        sxhi = _pack_idx_image((rows - HALF) if base >= HALF else dump)

        # pooling one-hot [128, NTC, 64]
        pool = np.zeros((P, NTC, G_GRAPHS), np.float16)
        for tt in range(NTC):
            gn = base + tt * P + np.arange(P)
            valid = gn < N
            pool[valid, tt, batch[gn[valid]]] = 1.0

        x_pad = np.zeros((NC_NODES, P), np.float32)
        hi_n = min(N, base + NC_NODES)
        if hi_n > base:
            x_pad[: hi_n - base] = np.asarray(x[base:hi_n], np.float32)

        in_maps.append({
            "x0T": np.ascontiguousarray(x_pad.T).astype(np.float16),
            "xlidxlo": np.concatenate(lo_imgs, axis=1),
            "xlidxhi": np.concatenate(hi_imgs, axis=1),
            "oh": np.concatenate(oh_cols, axis=1),
            "ohT": np.concatenate(ohT_cols, axis=1),
            "sxlo": sxlo,
            "sxhi": sxhi,
            "ident": ident,
            "attr0": att_reps[0], "attr1": att_reps[1], "attr2": att_reps[2],
            "wlr0": wlrs[0], "wlr1": wlrs[1], "wlr2": wlrs[2],
            "pooloh": pool,
        })

    return dict(CH=CH, CT=CT), in_maps


def _build(meta):
    CH, CT = meta["CH"], meta["CT"]
    NCH = NTC * CT  # total chunk columns per core
    nc = bacc.Bacc(
        get_trn_type() or "TRN2",
        target_bir_lowering=False,
        debug=False,
        num_devices=CORES,
        dynamic_dma_scratch_size=32768,   # 2048-descriptor SWDGE ring
    )
    inp = {}
    for name, shape, dt in [
        ("x0T", [P, NC_NODES], f16),
        ("xlidxlo", [P, NTC * CH * 8], i16),
        ("xlidxhi", [P, NTC * CH * 8], i16),
        ("oh", [P, NCH, P], f8),
        ("ohT", [P, NCH, P], f8),
        ("sxlo", [P, NC_NODES // 16], i16),
        ("sxhi", [P, NC_NODES // 16], i16),
        ("ident", [P, P], f8),
        ("attr0", [P, P], f16), ("attr1", [P, P], f16), ("attr2", [P, P], f16),
        ("wlr0", [P, 256], f16), ("wlr1", [P, 256], f16), ("wlr2", [P, 256], f16),
        ("pooloh", [P, NTC, G_GRAPHS], f16),
    ]:
        inp[name] = nc.dram_tensor(name, shape, dt, kind="ExternalInput")

    pooled = nc.dram_tensor("pooled", [G_GRAPHS, G_GRAPHS], f32,
                            kind="ExternalOutput")

    DIST = os.environ.get("GAT_DIST", "allgather")
    if DIST == "scatter":
        # HALF real rows + 128 dump rows for the off-half scatter call
        xl_lo_t = [nc.dram_tensor(f"xl_lo{l}", [HALF + P, P], f16,
                                  addr_space="Shared") for l in range(3)]
        xl_hi_t = [nc.dram_tensor(f"xl_hi{l}", [HALF + P, P], f16,
                                  addr_space="Shared") for l in range(3)]
        xl_lo = lambda l: xl_lo_t[l][0:HALF + P, :]  # noqa: E731
        xl_hi = lambda l: xl_hi_t[l][0:HALF + P, :]  # noqa: E731
    else:
        xl_own = nc.dram_tensor("xl_own", [NC_NODES, P], f16)
        xl_full = [nc.dram_tensor(f"xl_full{l}", [NP_, P], f16,
                                  addr_space="Shared") for l in range(3)]
        xl_lo = lambda l: xl_full[l][0:HALF, :]      # noqa: E731
        xl_hi = lambda l: xl_full[l][HALF:NP_, :]    # noqa: E731
    xn_own = nc.dram_tensor("xn_own", [NC_NODES, P], f16)
    flag_own = nc.dram_tensor("flag_own", [1, P], f16)
    flag_all = nc.dram_tensor("flag_all", [CORES, P], f16, addr_space="Shared")

    n_layers = int(os.environ.get("GAT_LAYERS", "3"))
    max_groups = int(os.environ.get("GAT_MAXG", "999"))
    dbg = os.environ.get("GAT_DEBUG")
    dbg_out = {}
    if dbg:
        for l in range(3):
            dbg_out[f"xlo{l}"] = nc.dram_tensor(
                f"dbg_xlo{l}", [HALF, P], f16, kind="ExternalOutput")
        for l in range(2):
            dbg_out[f"xn{l}"] = nc.dram_tensor(
                f"dbg_xn{l}", [NC_NODES, P], f16, kind="ExternalOutput")

    STRIP = 2  # tiles per phase-A' strip ([P, STRIP, 256] f32 PSUM)
    BATCH = 8  # chunks per PSUM u-batch (2 PSUM banks per buffer)

    with tile.TileContext(nc) as tc:
        with (
            tc.tile_pool(name="const", bufs=1) as cpool,
            tc.tile_pool(name="stage", bufs=1) as spool,
            tc.tile_pool(name="strip", bufs=3) as stpool,
            tc.tile_pool(name="edge", bufs=2) as epool,
            tc.tile_pool(name="small", bufs=3) as smpool,
            tc.tile_pool(name="psA", bufs=1, space="PSUM") as psA,
            tc.tile_pool(name="psU", bufs=2, space="PSUM") as psU,
            tc.tile_pool(name="psS", bufs=2, space="PSUM") as psS,
            tc.tile_pool(name="psP", bufs=1, space="PSUM") as psP,
        ):
            ident_t = cpool.tile([P, P], f8)
            nc.sync.dma_start(out=ident_t[:], in_=inp["ident"][:])
            pool_t = cpool.tile([P, NTC, G_GRAPHS], f16)
            nc.sync.dma_start(out=pool_t[:], in_=inp["pooloh"][:])
            sxlo_t = cpool.tile([P, NC_NODES // 16], i16)
            nc.sync.dma_start(out=sxlo_t[:], in_=inp["sxlo"][:])
            sxhi_t = cpool.tile([P, NC_NODES // 16], i16)
            nc.sync.dma_start(out=sxhi_t[:], in_=inp["sxhi"][:])

            pool_psum = psP.tile([G_GRAPHS, G_GRAPHS], f32, space="PSUM")

            # persistent per-layer state
            xr_sb = spool.tile([P, NTC, P], f16, tag="xr_sb")
            stg_xl = spool.tile([P, NTC, P], f16, tag="stg_xl")
            stg_xn = spool.tile([P, NTC, P], f16, tag="stg_xn")

            for l in range(n_layers):
                Hh = H_L[l]
                W = W_L[l]
                CW = W // Hh
                wlr_t = cpool.tile([P, 256], f16, tag="wlr")
                nc.sync.dma_start(out=wlr_t[:], in_=inp[f"wlr{l}"][:])
                att_t = cpool.tile([P, P], f16, tag="att")
                nc.sync.dma_start(out=att_t[:], in_=inp[f"attr{l}"][:])

                # ---- phase A': own-shard transforms ----
                for j0 in range(0, NTC, STRIP):
                    w_ = min(STRIP, NTC - j0)
                    if l == 0:
                        xs = inp["x0T"][:, j0 * P:(j0 + w_) * P]
                        xs_t = stpool.tile([P, w_ * P], f16, tag="xstrip")
                        nc.sync.dma_start(out=xs_t[:], in_=xs)
                    else:
                        xs_t = stpool.tile([P, w_ * P], f16, tag="xstrip")
                        nc.sync.dma_start_transpose(
                            out=xs_t[:], in_=xn_own[j0 * P:(j0 + w_) * P, :])
                    ps = psA.tile([P, STRIP, 2 * P], f32, space="PSUM",
                                  tag="psa")
                    for j in range(w_):
                        nc.tensor.matmul(
                            out=ps[:, j, :W], lhsT=xs_t[:, j * P:(j + 1) * P],
                            rhs=wlr_t[:, :W], start=True, stop=True)
                        nc.tensor.matmul(
                            out=ps[:, j, P:P + W],
                            lhsT=xs_t[:, j * P:(j + 1) * P],
                            rhs=wlr_t[:, 128:128 + W], start=True, stop=True)
                    nc.scalar.copy(out=stg_xl[:, j0:j0 + w_, :W],
                                   in_=ps[:, :w_, :W])
                    nc.scalar.copy(out=xr_sb[:, j0:j0 + w_, :W],
                                   in_=ps[:, :w_, P:P + W])

                if DIST == "scatter":
                    # scatter own xl rows into the shared halves (one live,
                    # one dead per core, chosen by the idx data)
                    nc.gpsimd.dma_scatter_add(
                        out_ap=xl_lo(l), in_ap=stg_xl[:], idxs_ap=sxlo_t[:],
                        num_idxs=NC_NODES, num_idxs_reg=NC_NODES, elem_size=P)
                    nc.gpsimd.dma_scatter_add(
                        out_ap=xl_hi(l), in_ap=stg_xl[:], idxs_ap=sxhi_t[:],
                        num_idxs=NC_NODES, num_idxs_reg=NC_NODES, elem_size=P)

                    # barrier: readback own writes -> flag -> AllGather
                    fb = smpool.tile([1, P], f16, tag="fb")
                    nc.sync.dma_start(out=fb[0:1, 0:64],
                                      in_=xl_lo(l)[0:1, 0:64])
                    nc.sync.dma_start(out=fb[0:1, 64:128],
                                      in_=xl_hi(l)[0:1, 0:64])
                    nc.sync.dma_start(out=flag_own[:], in_=fb[:])
                    nc.gpsimd.collective_compute(
                        "AllGather", mybir.AluOpType.bypass,
                        replica_groups=[list(range(CORES))],
                        ins=[flag_own[:]], outs=[flag_all[:]])
                else:
                    # write own xl shard to local DRAM, AllGather into the
                    # shared full tensor (natural node-row order)
                    nc.sync.dma_start(
                        out=xl_own[:].rearrange("(t p) f -> p t f", p=P),
                        in_=stg_xl[:])
                    nc.gpsimd.collective_compute(
                        "AllGather", mybir.AluOpType.bypass,
                        replica_groups=[list(range(CORES))],
                        ins=[xl_own[:]], outs=[xl_full[l][:]])

                if dbg:
                    for t0_ in range(0, HALF, 7 * P):
                        tmp = stpool.tile([P, 7, P], f16, tag="dbgcp")
                        nc.sync.dma_start(
                            out=tmp[:],
                            in_=xl_lo(l)[t0_:t0_ + 7 * P, :].rearrange(
                                "(c p) f -> p c f", p=P))
                        nc.sync.dma_start(
                            out=dbg_out[f"xlo{l}"][t0_:t0_ + 7 * P, :]
                            .rearrange("(c p) f -> p c f", p=P), in_=tmp[:])

                # ---- phase B: edge processing per group ----
                for gidx, (gi, g) in enumerate(_groups()):
                    if gidx >= max_groups:
                        continue
                    nch = g * CH           # chunks per stream
                    nch2 = 2 * nch
                    nlo_i = nch * P        # gather slots per stream
                    c_lo = gi * CH * 8     # idx-image col offset
                    col0 = gi * CT         # oh/ohT chunk col offset

                    ilo = smpool.tile([P, nlo_i // 16], i16, tag="ilo")
                    nc.sync.dma_start(
                        out=ilo[:], in_=inp["xlidxlo"][:, c_lo:c_lo + nlo_i // 16])
                    ihi = smpool.tile([P, nlo_i // 16], i16, tag="ihi")
                    nc.sync.dma_start(
                        out=ihi[:], in_=inp["xlidxhi"][:, c_lo:c_lo + nlo_i // 16])
                    oh_t = epool.tile([P, nch2, P], f8, tag="oh")
                    nc.sync.dma_start(
                        out=oh_t[:], in_=inp["oh"][:, col0:col0 + nch2, :])
                    ohT_t = epool.tile([P, nch2, P], f8, tag="ohT")
                    for o0 in range(0, nch2, BATCH):
                        on = min(BATCH, nch2 - o0)
                        nc.sync.dma_start(
                            out=ohT_t[:, o0:o0 + on, :],
                            in_=inp["ohT"][:, col0 + o0:col0 + o0 + on, :])

                    # per-call sub-tiles: each 1024-idx gather lands in its
                    # own tile so downstream batches start after the FIRST
                    # call instead of the last
                    def gathers(sname, in_ap, idx_t, slots):
                        subs, k, ci = [], 0, 0
                        while k < slots:
                            n = min(GMAX, slots - k)
                            st = epool.tile([P, n // P, P], f16,
                                            tag=f"xlg{sname}{ci}")
                            nc.gpsimd.dma_gather(
                                out_ap=st[:], in_ap=in_ap,
                                idxs_ap=idx_t[:, k // 16:(k + n) // 16],
                                num_idxs=n, num_idxs_reg=n, elem_size=P)
                            subs.append((st, k // P, n // P))
                            k += n
                            ci += 1
                        return subs

                    sub_lo = gathers("lo", xl_lo(l), ilo, nlo_i)
                    sub_hi = gathers("hi", xl_hi(l), ihi, nlo_i)

                    # u = xr[tloc] + xl_src  (PSUM), leaky-relu -> L
                    # per-chunk one-hot expands of xr, then ONE batched
                    # identity matmul accumulates the gathered xl rows
                    L_t = epool.tile([P, nch2, P], f16, tag="L")
                    for s, subs in ((0, sub_lo), (1, sub_hi)):
                        for xt, c0, ncc in subs:
                            for b0 in range(0, ncc, BATCH):
                                nb = min(BATCH, ncc - b0)
                                psu = psU.tile([P, BATCH, P], f32,
                                               space="PSUM", tag="psu")
                                for k in range(nb):
                                    cc = c0 + b0 + k
                                    cid = s * nch + cc
                                    tt = gi + cc // CH
                                    nc.tensor.matmul(
                                        out=psu[:, k, :W],
                                        lhsT=ohT_t[:, cid, :],
                                        rhs=xr_sb[:, tt, :W],
                                        start=True, stop=False)
                                    nc.tensor.matmul(
                                        out=psu[:, k, :W], lhsT=ident_t[:],
                                        rhs=xt[:, b0 + k, :W],
                                        start=False, stop=True)
                                nc.scalar.activation(
                                    out=L_t[:, s * nch + c0 + b0:
                                            s * nch + c0 + b0 + nb, :W],
                                    in_=psu[:, :nb, :W],
                                    func=mybir.ActivationFunctionType.Prelu,
                                    alpha=NEG)

                    # scores: L *= att ; tree-reduce over cw
                    nc.vector.tensor_tensor(
                        out=L_t[:, :, :W], in0=L_t[:, :, :W],
                        in1=att_t[:, :W].unsqueeze(1).broadcast_to(
                            [P, nch2, W]),
                        op=mybir.AluOpType.mult)
                    tree = epool.tile([P, nch2, P // 2], f16, tag="tree")
                    Lv = L_t[:, :, :W].rearrange("p c (w h) -> p c w h", h=Hh)
                    tv = tree[:, :, :W // 2].rearrange(
                        "p c (w h) -> p c w h", h=Hh)
                    half = CW // 2
                    nc.vector.tensor_tensor(
                        out=tv[:, :, :half, :], in0=Lv[:, :, :half, :],
                        in1=Lv[:, :, half:, :], op=mybir.AluOpType.add)
                    while half > 1:
                        q = half // 2
                        nc.vector.tensor_tensor(
                            out=tv[:, :, :q, :], in0=tv[:, :, :q, :],
                            in1=tv[:, :, q:half, :], op=mybir.AluOpType.add)
                        half = q
                    # w values and alpha
                    w_t = epool.tile([P, nch2, P + 4], f16, tag="w")
                    nc.scalar.activation(
                        out=w_t[:, :, W:W + Hh], in_=tv[:, :, 0, :],
                        func=mybir.ActivationFunctionType.Exp)
                    a_b = w_t[:, :, W:W + Hh].unsqueeze(2).broadcast_to(
                        [P, nch2, CW, Hh])
                    for s, subs in ((0, sub_lo), (1, sub_hi)):
                        for xt, c0, ncc in subs:
                            nc.vector.tensor_tensor(
                                out=w_t[:, s * nch + c0:s * nch + c0 + ncc,
                                        :W].rearrange(
                                    "p c (w h) -> p c w h", h=Hh),
                                in0=xt[:, :, :W].rearrange(
                                    "p c (w h) -> p c w h", h=Hh),
                                in1=a_b[:, s * nch + c0:s * nch + c0 + ncc],
                                op=mybir.AluOpType.mult)

                    # scatter per tile
                    for tt in range(g):
                        t = gi + tt
                        ps = psS.tile([P, P + 4], f32, space="PSUM", tag="pss")
                        for cix in range(CT):
                            s, cc = divmod(cix, CH)
                            cid = s * nch + tt * CH + cc
                            nc.tensor.matmul(
                                out=ps[:, :W + Hh],
                                lhsT=oh_t[:, cid, :],
                                rhs=w_t[:, cid, :W + Hh],
                                start=(cix == 0), stop=(cix == CT - 1))
                        den = smpool.tile([P, 4], f32, tag="den")
                        nc.vector.tensor_scalar_max(
                            out=den[:, :Hh], in0=ps[:, W:W + Hh], scalar1=1e-30)
                        rec = smpool.tile([P, 4], f32, tag="rec")
                        nc.vector.reciprocal(out=rec[:, :Hh], in_=den[:, :Hh])
                        t1 = smpool.tile([P, P], f16, tag="t1")
                        nc.vector.tensor_tensor(
                            out=t1[:, :W].rearrange("p (w h) -> p w h", h=Hh),
                            in0=ps[:, :W].rearrange("p (w h) -> p w h", h=Hh),
                            in1=rec[:, :Hh].unsqueeze(1).broadcast_to(
                                [P, CW, Hh]),
                            op=mybir.AluOpType.mult)
                        if l < 2:
                            nc.scalar.activation(
                                out=stg_xn[:, t, :], in_=t1[:],
                                func=mybir.ActivationFunctionType.Prelu,
                                alpha=NEG)
                        else:
                            xnm = smpool.tile([P, G_GRAPHS], f16, tag="xnm2")
                            nc.scalar.activation(
                                out=xnm[:], in_=t1[:, :G_GRAPHS],
                                func=mybir.ActivationFunctionType.Prelu,
                                alpha=NEG)
                            nc.tensor.matmul(
                                out=pool_psum[:],
                                lhsT=pool_t[:, t, :], rhs=xnm[:],
                                start=(t == 0), stop=(t == NTC - 1))

                if l < 2:
                    nc.sync.dma_start(
                        out=xn_own[:].rearrange("(t p) f -> p t f", p=P),
                        in_=stg_xn[:])
                    if dbg:
                        for t0_ in range(0, NTC, 7):
                            tmp = stpool.tile([P, 7, P], f16, tag="dbgcp")
                            nc.sync.dma_start(
                                out=tmp[:],
                                in_=xn_own[t0_ * P:(t0_ + 7) * P, :].rearrange(
                                    "(t p) f -> p t f", p=P))
                            nc.sync.dma_start(
                                out=dbg_out[f"xn{l}"][t0_ * P:(t0_ + 7) * P, :]
                                .rearrange("(t p) f -> p t f", p=P),
                                in_=tmp[:])

            pool_sb = smpool.tile([G_GRAPHS, G_GRAPHS], f32, tag="poolsb")
            if n_layers == 3 and max_groups >= len(_groups()):
                nc.vector.tensor_copy(out=pool_sb[:], in_=pool_psum[:])
            else:
                nc.vector.memset(pool_sb[:], 0.0)
            nc.sync.dma_start(out=pooled[:], in_=pool_sb[:])

    nc.finalize()
    return nc


def kernel(**inputs):
    x = np.asarray(inputs["x"])
    edge_index = np.asarray(inputs["edge_index"])
    batch = np.asarray(inputs["batch"])
    params = []
    for l in range(3):
        params.append((np.asarray(inputs[f"Wl{l}"]),
                       np.asarray(inputs[f"Wr{l}"]),
                       np.asarray(inputs[f"att{l}"])))
        b = np.asarray(inputs[f"b{l}"])
        assert np.all(b == 0), "nonzero bias not supported"

    meta, in_maps = _preprocess(x, edge_index, batch, params)

    key = ("nc", meta["CH"])
    if key not in _CACHE:
        _CACHE[key] = _build(meta)
    nc = _CACHE[key]

    try:
        res = run_bass_kernel_spmd(
            nc, in_maps, core_ids=list(range(CORES)),
            trace=bool(os.environ.get("GAT_TRACE")))
    except ModuleNotFoundError:
        res = run_bass_kernel_spmd(nc, in_maps, core_ids=list(range(CORES)))
    kernel._last_result = res

    pooled = np.zeros((G_GRAPHS, G_GRAPHS), np.float64)
    for c in range(CORES):
        pooled += res.results[c]["pooled"].astype(np.float64)
    cnt = np.bincount(batch, minlength=G_GRAPHS).astype(np.float64)
    out = pooled / np.maximum(cnt, 1.0)[:, None]

    if os.environ.get("GAT_DEBUG"):
        kernel._debug = res
    return out.astype(np.float32)
